# revision 27
# baseline (speedup 1.0000x reference)
"""Trainium2 Bass kernel for grouped-attention MoE routing.

Math (derived from the nn.Module):
  gate  = softmax(mlp(maxpool(conv(x))) + mlp(avgpool(conv(x))))      (B,45)
  sel   = sorted(top22(mean_b gate))                                  (22,)
  Per expert e with u = x[:, sel[e], :]:
    energy[l,m] = (a_e*u_l + g_e) * u_m   (rank-1; scalars a,g from weights)
    attn = softmax_m(energy);  s_l = sum_m u_m attn[l,m]
    y_l  = P_e*s_l + Q_e;      A[:,sel[e],:] = y * gate[:,sel[e]]
  G = x * A (flat);  return (G, A_flat)

Key optimization: with k = a*u_l + g, the softmax row sums are
  den_l = sum_m e^{k u_m},  num_l = sum_m u_m e^{k u_m}.
Approximating e^z by a degree-J Chebyshev fit P(z) = sum_j d_j z^j on the
realized z-range turns both into polynomials in k with power-sum
coefficients: den = sum_j d_j k^j S_j, num = sum_j d_j k^j S_{j+1}, where
S_j = sum_m u_m^j.  This replaces the O(L^2) energy tensor with O(L*J)
work (J=6 gives ~1e-3 end-to-end error vs the 2e-2 gate).

Layout: fp16 everywhere on the elementwise path with e (expert) innermost
so every scalar_tensor_tensor op hits the DVE 4x perf mode; power sums via
a pairwise tree reduce; Horner evaluation of num|den jointly (num and den
share the multiply-by-k steps and the immediate d_j coefficients once u is
pre-scaled by 1/2 on the host).

Strategy: pure data parallel over batch on 8 cores; two launches with the
45-float routing reduction mediated on host (equivalent of the all-reduce).
"""

import numpy as np
import ml_dtypes
from contextlib import ExitStack

import bass_rust
import concourse.bass as bass
import concourse.mybir as mybir
import concourse.tile as tile
from concourse.bass_utils import run_bass_kernel_spmd

_MULTIWAIT_OK = ("InstNoOp", "InstAllEngineBarrier",
                 "InstEventSemaphore", "InstUnconditionalBranch")


def legalize_sync_waits(nc):
    """walrus codegen on this stack rejects >1 sync wait on most
    instructions; hoist extra waits onto same-engine NoOps."""
    for func in nc.m.functions:
        for block in func.blocks:
            il = block.instructions
            out = []
            for inst in il:
                tname = type(inst).__name__
                si = getattr(inst, "sync_info", None)
                waits = list(si.on_wait) if si is not None else []
                if tname not in _MULTIWAIT_OK and len(waits) > 1:
                    for k, w in enumerate(waits):
                        nop = mybir.InstNoOp(
                            name=f"{inst.name}-synop{k}", ins=[], outs=[])
                        nop.engine = inst.engine
                        nop.sync_info = bass_rust.SyncInfo(
                            on_wait=[w], on_update=[])
                        out.append(nop)
                    inst.sync_info = bass_rust.SyncInfo(
                        on_wait=[], on_update=list(inst.sync_info.on_update))
                out.append(inst)
            il.clear()
            il.extend(out)


B, C, L, E = 8192, 45, 21, 22
NCORES = 8
BC = B // NCORES          # rows per core
P = 128                   # SBUF partitions
NT = BC // P              # batch tiles per core
CL = C * L                # 945
EL = E * L                # 462
J = 5                     # exp-approx polynomial degree
F32 = mybir.dt.float32
F16 = mybir.dt.float16
BF16 = mybir.dt.bfloat16
AF = mybir.ActivationFunctionType
ALU = mybir.AluOpType
AX = mybir.AxisListType
BYP = ALU.bypass

# channel groups for the gating conv matmul: 8 groups of <=6 channels
GROUPS = [list(range(g, min(g + 6, C))) for g in range(0, C, 6)]
NG = len(GROUPS)          # 8
GROWS = 127               # rows per chunk in the host-packed transposed x


def _ap(base, extra_free):
    """Custom free-dim access pattern on a tile slice: keep the partition
    dim of `base`, replace the free dims."""
    return bass.AP(tensor=base.tensor, offset=base.offset,
                   ap=[base.ap[0]] + extra_free)


def _dram_ap(dram, offset, ap):
    base = dram[:, :] if len(dram.shape) > 1 else dram[:]
    return bass.AP(tensor=base.tensor, offset=base.offset + offset, ap=ap)


def build_gate_program():
    """Gating network. x arrives host-transposed as 8 row-chunks of 127
    (6 channels x 21 taps + a ones row for bias), bf16.  Conv + avg-pool
    ride the PE as block-diagonal matmuls into a single bf16 PSUM bank
    (double-buffered); max-pool on DVE; the MLP runs transposed (bias via
    per-partition activation bias) with per-branch PSUM banks so the two
    branches and adjacent tiles overlap; softmax skips the max-subtract
    (|z| <= 2 by construction). Output gate in fp16."""
    nc = bass.Bass()
    # packed constants: one bf16 block [wcat | wav | w1a | w2a | ident]
    # (column offsets 0/945/990/1015/1060) and one f32 block [b1c | b2c]
    NCB = CL + C + 25 + C + P
    xg = nc.declare_dram_parameter("xg", [NT * GROWS, NG * P], BF16,
                                   isOutput=False)
    cstb = nc.declare_dram_parameter("cstb", [P, NCB], BF16, isOutput=False)
    cstf = nc.declare_dram_parameter("cstf", [C, 2], F32, isOutput=False)
    gate_o = nc.declare_dram_parameter("gate", [BC, C], F16, isOutput=True)

    # per-group geometry: (chunk row base, data rows, out-col base, n chans)
    geo = []
    cb = 0
    for g, chans in enumerate(GROUPS):
        nch = len(chans)
        geo.append((g * GROWS, nch * L, cb, nch))
        cb += nch * L

    with tile.TileContext(nc) as tc, ExitStack() as ctx:
        singles = ctx.enter_context(tc.tile_pool(name="singles", bufs=1))
        xs = ctx.enter_context(tc.tile_pool(name="xs", bufs=3))
        cp = ctx.enter_context(tc.tile_pool(name="cp", bufs=2))
        hw = ctx.enter_context(tc.tile_pool(name="hw", bufs=2))
        sm = ctx.enter_context(tc.tile_pool(name="sm", bufs=3))
        ps = ctx.enter_context(tc.tile_pool(name="ps", bufs=2, space="PSUM"))
        psm = ctx.enter_context(tc.tile_pool(name="psm", bufs=1, space="PSUM"))

        # PE-read consts funnel through DVE (one-wait matmul constraint);
        # warm-up transpose advances PE's observed DVE clock past them.
        def dve_const(dram, p, n, dt):
            raw = singles.tile([p, n], dt, name="raw_" + dram.name)
            nc.sync.dma_start(out=raw, in_=dram[:, :])
            t = singles.tile([p, n], dt, name="sb_" + dram.name)
            nc.vector.tensor_copy(out=t, in_=raw)
            return t

        sb_cb = dve_const(cstb, P, NCB, BF16)
        sb_cf = dve_const(cstf, C, 2, F32)
        sb_wcat = sb_cb[0:GROWS, 0:CL]
        sb_wav = sb_cb[0:GROWS, CL:CL + C]
        sb_w1a = sb_cb[0:C, CL + C:CL + C + 25]
        sb_w2a = sb_cb[0:25, CL + C + 25:CL + 2 * C + 25]
        sb_id = sb_cb[0:P, CL + 2 * C + 25:NCB]
        sb_b1c = sb_cf[0:25, 0:1]
        sb_b2c = sb_cf[0:C, 1:2]
        ones_col = singles.tile([P, 1], BF16)
        nc.vector.memset(ones_col, 1.0)
        warm_ps = psm.tile([C, 4 * P], BF16, tag="hTq")
        nc.tensor.transpose(warm_ps[0:1, 0:P], ones_col, sb_id)

        QT = 4                      # tiles batched through one MLP pass
        for q in range(NT // QT):
            mxq = sm.tile([P, QT * 48], BF16, tag="mxq")
            avq = sm.tile([P, QT * C], BF16, tag="avq")
            for ti in range(QT):
                t = q * QT + ti
                # one DMA: 8 transposed chunks side by side (127, 8*128)
                xgt = xs.tile([GROWS, NG * P], BF16, tag="xgt")
                nc.sync.dma_start(
                    out=xgt[:, :],
                    in_=xg[t * GROWS:(t + 1) * GROWS, :])

                # conv + avg into one PSUM tile: [0:945) conv, [945:990) avg
                tp = ps.tile([P, CL + C], F32, tag="tp")
                for g, (rbase, rdata, cbase, nch) in enumerate(geo):
                    lhs = xgt[0:rdata + 1, g * P:(g + 1) * P]
                    nc.tensor.matmul(
                        tp[:, cbase:cbase + nch * L], lhs,
                        sb_cb[0:rdata + 1, cbase:cbase + nch * L],
                        start=True, stop=True)
                    cav = sum(len(ch) for ch in GROUPS[:g])
                    nc.tensor.matmul(tp[:, CL + cav:CL + cav + nch], lhs,
                                     sb_cb[0:rdata + 1, CL + cav:CL + cav + nch],
                                     start=True, stop=True)

                # avg copy (ACT); max-pool via three PSUM reduces (DVE)
                nc.scalar.activation(out=avq[:, ti * C:(ti + 1) * C],
                                     in_=tp[:, CL:CL + C], func=AF.Copy)
                nc.vector.tensor_reduce(
                    out=mxq[:, ti * 48:ti * 48 + C],
                    in_=_ap(tp[:, 0:CL], [[L, C], [1, L]]),
                    axis=AX.X, op=ALU.max)

            # quad MLP, branches merged as extra columns: one
            # matmul/tanh chain covers 4 tiles x {max, avg} as (.., 1024)
            hTq_ps = psm.tile([C, 2 * QT * P], BF16, tag="hTq")
            for ti in range(QT):
                nc.tensor.transpose(hTq_ps[:, ti * P:(ti + 1) * P],
                                    mxq[:, ti * 48:ti * 48 + C], sb_id)
                nc.tensor.transpose(
                    hTq_ps[:, (QT + ti) * P:(QT + ti + 1) * P],
                    avq[:, ti * C:(ti + 1) * C], sb_id)
            hTq = hw.tile([C, 2 * QT * P], BF16, tag="hTq")
            nc.vector.tensor_copy(out=hTq, in_=hTq_ps)
            pmlp = psm.tile([C, 2 * QT * P], F32, tag="pmlp")
            H = QT * P                      # 512-col halves: one PSUM bank
            nc.tensor.matmul(pmlp[0:25, 0:H], sb_w1a, hTq[:, 0:H],
                             start=True, stop=True)
            nc.tensor.matmul(pmlp[0:25, H:2 * H], sb_w1a, hTq[:, H:2 * H],
                             start=True, stop=True)
            t1Tq = hw.tile([25, 2 * QT * P], BF16, tag="t1Tq")
            nc.scalar.activation(out=t1Tq, in_=pmlp[0:25, :], func=AF.Tanh,
                                 bias=sb_b1c)
            nc.tensor.matmul(pmlp[:, 0:H], sb_w2a, t1Tq[:, 0:H],
                             start=True, stop=True)
            nc.tensor.matmul(pmlp[:, H:2 * H], sb_w2a, t1Tq[:, H:2 * H],
                             start=True, stop=True)
            zTq = hw.tile([C, 2 * QT * P], BF16, tag="zTq")
            nc.scalar.activation(out=zTq, in_=pmlp, func=AF.Tanh, bias=sb_b2c)

            # sum the max/avg halves; transpose back per tile; softmax
            zTsum = hw.tile([C, QT * P], BF16, tag="zTsum")
            nc.vector.tensor_add(out=zTsum, in0=zTq[:, 0:QT * P],
                                 in1=zTq[:, QT * P:2 * QT * P])
            zsq = psm.tile([P, QT * 48], BF16, tag="zsq")
            gtq = sm.tile([P, QT * 48], F16, tag="gtq")
            for ti in range(QT):
                t = q * QT + ti
                nc.tensor.transpose(zsq[:, ti * 48:ti * 48 + C],
                                    zTsum[:, ti * P:(ti + 1) * P],
                                    sb_cb[0:C, CL + 2 * C + 25:CL + 2 * C + 25 + C])
                eg = sm.tile([P, C], F16, tag=f"eg{ti}")
                ssum = sm.tile([P, 1], F32, tag=f"ssum{ti}")
                nc.scalar.activation(out=eg, in_=zsq[:, ti * 48:ti * 48 + C],
                                     func=AF.Exp, accum_out=ssum)
                rs = sm.tile([P, 1], F32, tag=f"rs{ti}")
                nc.vector.reciprocal(out=rs, in_=ssum)
                nc.vector.tensor_scalar_mul(out=gtq[:, ti * 48:ti * 48 + C],
                                            in0=eg, scalar1=rs)
            nc.scalar.dma_start(
                out=_dram_ap(gate_o, q * QT * P * C,
                             [[C, P], [P * C, QT], [1, C]]),
                in_=_ap(gtq[:, 0:], [[48, QT], [1, C]]))
    legalize_sync_waits(nc)
    return nc


def build_attn_program(dj):
    """Rank-1 attention via the polynomial trick.  The host ships, per
    row and in l-major fp16 (element (l,e) at l*22+e): u' = x_sel/2
    (halved so fp16 power sums cannot overflow), kap = a*x_sel + g, and
    the selected gate row.  dj[j] = cheb_j * 2^j are shared step
    immediates.

    den and the gated numerator M = gp*num/2 + gq*den (gp = 4P_e*gate,
    gq = 2Q_e*gate) are evaluated jointly by one Horner pass over a
    duplicated-expert axis eh=44: the step-j coefficient is the
    contiguous 44-wide slice [dj*S'_j | dj*(gp*S'_{j+1}+gq*S'_j)] of a
    prebuilt stack, broadcast over l; then 2*A = M/den and A*x follow.
    Only plain TensorScalarPtr/TensorCopy get DVE 2x/4x modes and walrus
    limits TSP to 2 free dims, so the hot loop is fp16 InstTensorTensor
    (2x_1p) with 3-free-dim access patterns; power sums use a pairwise
    tree over l on an (l, j, e)-interleaved power stack."""
    nc = bass.Bass()
    W3 = 2 * EL + E
    xsg = nc.declare_dram_parameter("xsg", [BC, W3], F16, isOutput=False)
    # packed broadcast constants [djv | p2v | qvv]
    NCC = (J + 1) * 2 * E + 2 * E
    cstc = nc.declare_dram_parameter("cstc", [NCC], F16, isOutput=False)
    o16 = nc.declare_dram_parameter("o16", [BC, 2 * EL], F16, isOutput=True)

    NJ = J + 1        # powers u'^1..u'^{J+1}
    ROW = NJ * E      # one l-row of the interleaved power stack

    with tile.TileContext(nc) as tc, ExitStack() as ctx:
        singles = ctx.enter_context(tc.tile_pool(name="singles", bufs=1))
        pstk = ctx.enter_context(tc.tile_pool(name="pstk", bufs=5))
        trp = ctx.enter_context(tc.tile_pool(name="trp", bufs=5))
        kp = ctx.enter_context(tc.tile_pool(name="kp", bufs=5))
        hp = ctx.enter_context(tc.tile_pool(name="hp", bufs=5))
        op = ctx.enter_context(tc.tile_pool(name="op", bufs=5))

        base = cstc[:]
        cB = singles.tile([P, NCC], F16, name="bc_cstc")
        nc.gpsimd.dma_start(
            out=cB, in_=bass.AP(tensor=base.tensor, offset=base.offset,
                                ap=[[0, P], [1, NCC]]))
        djB = cB[:, 0:(J + 1) * 2 * E]
        p2B = cB[:, NCC - 2 * E:NCC - E]
        qB = cB[:, NCC - E:NCC]

        for t in range(NT):
            ug = kp.tile([P, W3], F16, tag="ug")     # [u' | kap | gate_sel]
            nc.sync.dma_start(out=ug, in_=xsg[t * P:(t + 1) * P, :])
            u0 = ug[:, 0:EL]
            kapv = ug[:, EL:2 * EL]
            gst = ug[:, 2 * EL:W3]

            # interleaved power stack (l, j, e); ACT copies u' to slot 1
            pst = pstk.tile([P, L * ROW], F16, tag="pst")

            def slot(j):          # (l, e) view of power j
                return _ap(pst[:, (j - 1) * E:], [[ROW, L], [1, E]])

            if t == 0:
                nc.vector.tensor_copy(out=slot(1), in_=u0)
            else:
                nc.scalar.activation(out=slot(1), in_=u0, func=AF.Copy)

            # powers u'^2..u'^{J+1}: squares on ACT, odd muls on DVE/Pool
            nc.scalar.activation(out=slot(2), in_=u0, func=AF.Square)
            nc.vector.tensor_mul(out=slot(3), in0=slot(2), in1=slot(1))
            nc.scalar.activation(out=slot(4), in_=slot(2), func=AF.Square)
            if NJ >= 5:
                nc.gpsimd.tensor_mul(out=slot(5), in0=slot(3), in1=slot(2))
            if NJ >= 6:
                nc.scalar.activation(out=slot(6), in_=slot(3), func=AF.Square)
            if NJ >= 7:
                nc.gpsimd.tensor_mul(out=slot(7), in0=slot(6), in1=slot(1))

            # pairwise tree over l; (j,e) stays contiguous throughout, so
            # the final step writes S'_1.. straight into the S-stack
            Sp = trp.tile([P, (NJ + 1) * E], F16, tag="Sp")
            nc.gpsimd.memset(Sp[:, 0:E], float(L))
            t1 = trp.tile([P, 10 * ROW], F16, tag="t1")
            t2 = trp.tile([P, 5 * ROW], F16, tag="t2")
            t3 = trp.tile([P, 2 * ROW], F16, tag="t3")
            t4 = trp.tile([P, ROW], F16, tag="t4")
            t5 = trp.tile([P, ROW], F16, tag="t5")

            pR = lambda l0, n: _ap(pst[:, l0 * ROW:], [[ROW, n], [1, ROW]])
            tR = lambda tl, l0, n: _ap(tl[:, l0 * ROW:], [[ROW, n], [1, ROW]])
            nc.vector.tensor_add(out=tR(t1, 0, 10), in0=pR(0, 10),
                                 in1=pR(10, 10))
            nc.vector.tensor_add(out=tR(t2, 0, 5), in0=tR(t1, 0, 5),
                                 in1=tR(t1, 5, 5))
            nc.gpsimd.tensor_add(out=tR(t3, 0, 2), in0=tR(t2, 0, 2),
                                 in1=tR(t2, 2, 2))
            nc.gpsimd.tensor_add(out=tR(t4, 0, 1), in0=tR(t3, 0, 1),
                                 in1=tR(t3, 1, 1))
            nc.gpsimd.tensor_add(out=tR(t5, 0, 1), in0=tR(t4, 0, 1),
                                 in1=tR(t2, 4, 1))
            nc.gpsimd.tensor_add(out=_ap(Sp[:, E:], [[1, ROW]]),
                                 in0=tR(t5, 0, 1), in1=pR(20, 1))

            # coefficient pairs: [dj*S'_j | dj*(gp*S'_{j+1} + gq*S'_j)]
            gp = kp.tile([P, E], F16, tag="gp")
            nc.gpsimd.tensor_mul(out=gp, in0=gst, in1=p2B)
            gq = kp.tile([P, E], F16, tag="gq")
            nc.gpsimd.tensor_mul(out=gq, in0=gst, in1=qB)
            SS = trp.tile([P, (J + 1) * 2 * E], F16, tag="SS")
            tq = trp.tile([P, (J + 1) * E], F16, tag="tq")
            wJ = lambda tl, off: _ap(tl[:, off:], [[E, J + 1], [1, E]])
            wS = lambda off: _ap(SS[:, off:], [[2 * E, J + 1], [1, E]])
            wD = lambda off: _ap(djB[:, off:], [[2 * E, J + 1], [1, E]])
            gpB = _ap(gp[:, 0:E], [[0, J + 1], [1, E]])
            gqB = _ap(gq[:, 0:E], [[0, J + 1], [1, E]])
            nc.vector.tensor_mul(out=wS(0), in0=wJ(Sp, 0), in1=wD(0))
            nc.gpsimd.tensor_mul(out=wS(E), in0=wJ(Sp, E), in1=gpB)
            nc.gpsimd.tensor_mul(out=wJ(tq, 0), in0=wJ(Sp, 0), in1=gqB)
            nc.gpsimd.tensor_add(out=wS(E), in0=wS(E), in1=wJ(tq, 0))
            nc.gpsimd.tensor_mul(out=wS(E), in0=wS(E), in1=wD(E))

            # joint Horner on [den | M] with 3-free-dim TT ops
            ra = hp.tile([P, 2 * EL], F16, tag="ra")
            rb = hp.tile([P, 2 * EL], F16, tag="rb")
            f44 = lambda tl: _ap(tl[:, 0:], [[EL, 2], [E, L], [1, E]])
            cj = lambda j: _ap(SS[:, j * 2 * E:], [[E, 2], [0, L], [1, E]])
            kB = _ap(ug[:, EL:], [[0, 2], [E, L], [1, E]])
            cur, other = ra, rb
            nc.vector.tensor_mul(out=f44(cur), in0=cj(J), in1=kB)
            for j in range(J - 1, -1, -1):
                nc.vector.tensor_add(out=f44(other), in0=cj(j), in1=f44(cur))
                cur, other = other, cur
                if j > 0:
                    nc.vector.tensor_mul(out=f44(other), in0=f44(cur), in1=kB)
                    cur, other = other, cur

            # 2*A = M/den (host halves A on the way out); A*x = 2A * u'
            rd32 = op.tile([P, EL], F32, tag="rd32")
            nc.vector.reciprocal(out=rd32, in_=cur[:, 0:EL])
            ot = op.tile([P, 2 * EL], F16, tag="ot")
            eng = nc.vector if t == NT - 1 else nc.gpsimd
            eng.tensor_mul(out=ot[:, 0:EL], in0=cur[:, EL:2 * EL], in1=rd32)
            eng.tensor_mul(out=ot[:, EL:2 * EL], in0=ot[:, 0:EL], in1=u0)
            nc.scalar.dma_start(out=o16[t * P:(t + 1) * P, :], in_=ot)
    legalize_sync_waits(nc)
    return nc


def _gate_params(inputs):
    gc_w, gc_b = inputs["gc_w"], inputs["gc_b"]
    wbar = gc_w.mean(0)
    bbar = gc_b.mean()
    NCB = CL + C + 25 + C + P
    cstb = np.zeros((P, NCB), np.float32)
    cb = 0
    for g, chans in enumerate(GROUPS):
        nch = len(chans)
        for k, c in enumerate(chans):
            cstb[k * L:(k + 1) * L, cb + k * L:cb + (k + 1) * L] = gc_w.T
            cstb[k * L:(k + 1) * L, CL + c] = wbar
            cstb[nch * L, CL + c] = bbar
        cstb[nch * L, cb:cb + nch * L] = np.tile(gc_b, nch)
        cb += nch * L
    cstb[0:C, CL + C:CL + C + 25] = inputs["w1"].T
    cstb[0:25, CL + C + 25:CL + 2 * C + 25] = inputs["w2"].T
    cstb[:, CL + 2 * C + 25:NCB] = np.eye(P)
    cstf = np.zeros((C, 2), np.float32)
    cstf[0:25, 0] = inputs["b1"]
    cstf[0:C, 1] = inputs["b2"]
    return cstb.astype(ml_dtypes.bfloat16), cstf


_CACHE = {}


def kernel(**inputs):
    inputs = {k: np.ascontiguousarray(np.asarray(v)) for k, v in inputs.items()}
    x = inputs["x"].astype(np.float32)              # (B, C, L)
    bf = ml_dtypes.bfloat16
    cores = list(range(NCORES))

    # ---- launch 1: gate -------------------------------------------------
    cstb, cstf = _gate_params(inputs)
    # host-marshaled transposed x: 8 chunks of (6ch x 21 + ones row) x B
    xt = x.reshape(B, CL).T                          # (945, B)
    xgc = np.zeros((NG * GROWS, B), np.float32)
    cb = 0
    for g, chans in enumerate(GROUPS):
        nch = len(chans)
        xgc[g * GROWS:g * GROWS + nch * L] = xt[cb:cb + nch * L]
        xgc[g * GROWS + nch * L] = 1.0
        cb += nch * L
    # tile-major repack: per 128-row batch tile, the 8 chunks side by side
    # so each tile load is one contiguous (127, 1024) DMA
    xg = np.ascontiguousarray(
        xgc.reshape(NG, GROWS, NCORES, NT, P)       # (g, r, core, t, p)
        .transpose(2, 3, 1, 0, 4)                    # (core, t, r, g, p)
        .reshape(NCORES, NT * GROWS, NG * P)).astype(bf)

    if "gate" not in _CACHE:
        _CACHE["gate"] = build_gate_program()
    nc1 = _CACHE["gate"]
    maps1 = [{"xg": xg[i], "cstb": cstb, "cstf": cstf} for i in cores]
    r1 = run_bass_kernel_spmd(nc1, maps1, cores).results
    gate16 = np.concatenate([np.asarray(r["gate"]) for r in r1], 0)  # (B,45)

    # ---- routing (host-mediated all-reduce) -----------------------------
    mean_gate = gate16.astype(np.float64).mean(0)
    sel = np.sort(np.argsort(-mean_gate, kind="stable")[:E])

    # ---- launch 2: attention -------------------------------------------
    wq, bq = inputs["wq"], inputs["bq"]
    wk, bk = inputs["wk"], inputs["bk"]
    wv, bv = inputs["wv"], inputs["bv"]
    wo, bo = inputs["wo"], inputs["bo"]
    alpha = (wq * wk).sum(1).astype(np.float64)
    gamma = (bq * wk).sum(1).astype(np.float64)
    pv = (wo * wv).sum(1).astype(np.float64)
    qv = ((wo * bv).sum(1) + bo).astype(np.float64)

    xsel = x[:, sel, :]                              # (B, E, L)
    umax = float(np.abs(xsel).max())
    zm = (np.abs(alpha).max() * umax + np.abs(gamma).max()) * umax
    cheb = np.polynomial.chebyshev.Chebyshev.interpolate(
        np.exp, J, domain=[-zm, zm])
    dc = cheb.convert(kind=np.polynomial.Polynomial).coef
    dj = [float(dc[j] * (2.0 ** j)) for j in range(J + 1)]

    key = tuple(np.round(dj, 12))
    if _CACHE.get("attn_key") != key:
        _CACHE["attn"] = build_attn_program(dj)
        _CACHE["attn_key"] = key
    nc2 = _CACHE["attn"]

    xsg = np.empty((B, 2 * EL + E), np.float16)  # [u' | kap | gate] l-major
    xlm = np.ascontiguousarray(xsel.transpose(0, 2, 1).astype(np.float32))
    xsg[:, :EL] = (xlm * np.float32(0.5)).reshape(B, EL)
    xsg[:, EL:2 * EL] = (xlm * alpha.astype(np.float32)[None, None, :]
                         + gamma.astype(np.float32)[None, None, :]
                         ).reshape(B, EL)
    xsg[:, 2 * EL:] = gate16[:, sel]
    cstc = np.concatenate([
        np.repeat(np.asarray(dj), 2 * E).astype(np.float16),
        (4 * pv).astype(np.float16),
        (2 * qv).astype(np.float16)]).astype(np.float16)
    maps2 = [{"xsg": xsg[i * BC:(i + 1) * BC], "cstc": cstc}
             for i in cores]
    r2 = run_bass_kernel_spmd(nc2, maps2, cores).results
    o16 = np.concatenate([np.asarray(r["o16"]) for r in r2], 0)  # (B, 924)

    # ---- host unshard / scatter (device emits 2*A and A*x) -------------
    at = (o16[:, :EL].astype(np.float32) * 0.5).reshape(
        B, L, E).transpose(0, 2, 1)
    gt = o16[:, EL:].astype(np.float32).reshape(B, L, E).transpose(0, 2, 1)
    cols = (np.repeat(sel * L, L) + np.tile(np.arange(L), E))
    A_full = np.zeros((B, CL), np.float32)
    G_full = np.zeros((B, CL), np.float32)
    A_full[:, cols] = at.reshape(B, EL)
    G_full[:, cols] = gt.reshape(B, EL)
    return G_full, A_full


# revision 28
# speedup vs baseline: 1.1000x; 1.1000x over previous
"""Trainium2 Bass kernel for grouped-attention MoE routing.

Math (derived from the nn.Module):
  gate  = softmax(mlp(maxpool(conv(x))) + mlp(avgpool(conv(x))))      (B,45)
  sel   = sorted(top22(mean_b gate))                                  (22,)
  Per expert e with u = x[:, sel[e], :]:
    energy[l,m] = (a_e*u_l + g_e) * u_m   (rank-1; scalars a,g from weights)
    attn = softmax_m(energy);  s_l = sum_m u_m attn[l,m]
    y_l  = P_e*s_l + Q_e;      A[:,sel[e],:] = y * gate[:,sel[e]]
  G = x * A (flat);  return (G, A_flat)

Key optimization: with k = a*u_l + g, the softmax row sums are
  den_l = sum_m e^{k u_m},  num_l = sum_m u_m e^{k u_m}.
Approximating e^z by a degree-J Chebyshev fit P(z) = sum_j d_j z^j on the
realized z-range turns both into polynomials in k with power-sum
coefficients: den = sum_j d_j k^j S_j, num = sum_j d_j k^j S_{j+1}, where
S_j = sum_m u_m^j.  This replaces the O(L^2) energy tensor with O(L*J)
work (J=6 gives ~1e-3 end-to-end error vs the 2e-2 gate).

Layout: fp16 everywhere on the elementwise path with e (expert) innermost
so every scalar_tensor_tensor op hits the DVE 4x perf mode; power sums via
a pairwise tree reduce; Horner evaluation of num|den jointly (num and den
share the multiply-by-k steps and the immediate d_j coefficients once u is
pre-scaled by 1/2 on the host).

Strategy: pure data parallel over batch on 8 cores; two launches with the
45-float routing reduction mediated on host (equivalent of the all-reduce).
"""

import numpy as np
import ml_dtypes
from contextlib import ExitStack

import bass_rust
import concourse.bass as bass
import concourse.mybir as mybir
import concourse.tile as tile
from concourse.bass_utils import run_bass_kernel_spmd

_MULTIWAIT_OK = ("InstNoOp", "InstAllEngineBarrier",
                 "InstEventSemaphore", "InstUnconditionalBranch")


def legalize_sync_waits(nc):
    """walrus codegen on this stack rejects >1 sync wait on most
    instructions; hoist extra waits onto same-engine NoOps."""
    for func in nc.m.functions:
        for block in func.blocks:
            il = block.instructions
            out = []
            for inst in il:
                tname = type(inst).__name__
                si = getattr(inst, "sync_info", None)
                waits = list(si.on_wait) if si is not None else []
                if tname not in _MULTIWAIT_OK and len(waits) > 1:
                    for k, w in enumerate(waits):
                        nop = mybir.InstNoOp(
                            name=f"{inst.name}-synop{k}", ins=[], outs=[])
                        nop.engine = inst.engine
                        nop.sync_info = bass_rust.SyncInfo(
                            on_wait=[w], on_update=[])
                        out.append(nop)
                    inst.sync_info = bass_rust.SyncInfo(
                        on_wait=[], on_update=list(inst.sync_info.on_update))
                out.append(inst)
            il.clear()
            il.extend(out)


B, C, L, E = 8192, 45, 21, 22
NCORES = 8
BC = B // NCORES          # rows per core
P = 128                   # SBUF partitions
NT = BC // P              # batch tiles per core
CL = C * L                # 945
EL = E * L                # 462
J = 5                     # exp-approx polynomial degree
F32 = mybir.dt.float32
F16 = mybir.dt.float16
BF16 = mybir.dt.bfloat16
AF = mybir.ActivationFunctionType
ALU = mybir.AluOpType
AX = mybir.AxisListType
BYP = ALU.bypass

# channel groups for the gating conv matmul: 8 groups of <=6 channels
GROUPS = [list(range(g, min(g + 6, C))) for g in range(0, C, 6)]
NG = len(GROUPS)          # 8
GROWS = 127               # rows per chunk in the host-packed transposed x


def _ap(base, extra_free):
    """Custom free-dim access pattern on a tile slice: keep the partition
    dim of `base`, replace the free dims."""
    return bass.AP(tensor=base.tensor, offset=base.offset,
                   ap=[base.ap[0]] + extra_free)


def _dram_ap(dram, offset, ap):
    base = dram[:, :] if len(dram.shape) > 1 else dram[:]
    return bass.AP(tensor=base.tensor, offset=base.offset + offset, ap=ap)


def build_gate_program():
    """Gating network. x arrives host-transposed as 8 row-chunks of 127
    (6 channels x 21 taps + a ones row for bias), bf16.  Conv + avg-pool
    ride the PE as block-diagonal matmuls into a single bf16 PSUM bank
    (double-buffered); max-pool on DVE; the MLP runs transposed (bias via
    per-partition activation bias) with per-branch PSUM banks so the two
    branches and adjacent tiles overlap; softmax skips the max-subtract
    (|z| <= 2 by construction). Output gate in fp16."""
    nc = bass.Bass()
    # packed constants: one bf16 block [wcat | wav | w1a | w2a | ident]
    # (column offsets 0/945/990/1015/1060) and one f32 block [b1c | b2c]
    NCB = CL + C + 25 + C + P
    xg = nc.declare_dram_parameter("xg", [NT * GROWS, NG * P], BF16,
                                   isOutput=False)
    cstb = nc.declare_dram_parameter("cstb", [P, NCB], BF16, isOutput=False)
    cstf = nc.declare_dram_parameter("cstf", [C, 2], F32, isOutput=False)
    gate_o = nc.declare_dram_parameter("gate", [BC, C], F16, isOutput=True)

    # per-group geometry: (chunk row base, data rows, out-col base, n chans)
    geo = []
    cb = 0
    for g, chans in enumerate(GROUPS):
        nch = len(chans)
        geo.append((g * GROWS, nch * L, cb, nch))
        cb += nch * L

    with tile.TileContext(nc) as tc, ExitStack() as ctx:
        singles = ctx.enter_context(tc.tile_pool(name="singles", bufs=1))
        xs = ctx.enter_context(tc.tile_pool(name="xs", bufs=3))
        cp = ctx.enter_context(tc.tile_pool(name="cp", bufs=2))
        hw = ctx.enter_context(tc.tile_pool(name="hw", bufs=2))
        sm = ctx.enter_context(tc.tile_pool(name="sm", bufs=3))
        ps = ctx.enter_context(tc.tile_pool(name="ps", bufs=2, space="PSUM"))
        psm = ctx.enter_context(tc.tile_pool(name="psm", bufs=1, space="PSUM"))

        # PE-read consts funnel through DVE (one-wait matmul constraint);
        # warm-up transpose advances PE's observed DVE clock past them.
        def dve_const(dram, p, n, dt):
            raw = singles.tile([p, n], dt, name="raw_" + dram.name)
            nc.sync.dma_start(out=raw, in_=dram[:, :])
            t = singles.tile([p, n], dt, name="sb_" + dram.name)
            nc.vector.tensor_copy(out=t, in_=raw)
            return t

        sb_cb = dve_const(cstb, P, NCB, BF16)
        sb_cf = dve_const(cstf, C, 2, F32)
        sb_wcat = sb_cb[0:GROWS, 0:CL]
        sb_wav = sb_cb[0:GROWS, CL:CL + C]
        sb_w1a = sb_cb[0:C, CL + C:CL + C + 25]
        sb_w2a = sb_cb[0:25, CL + C + 25:CL + 2 * C + 25]
        sb_id = sb_cb[0:P, CL + 2 * C + 25:NCB]
        sb_b1c = sb_cf[0:25, 0:1]
        sb_b2c = sb_cf[0:C, 1:2]
        ones_col = singles.tile([P, 1], BF16)
        nc.vector.memset(ones_col, 1.0)
        warm_ps = psm.tile([C, 4 * P], BF16, tag="hTq")
        nc.tensor.transpose(warm_ps[0:1, 0:P], ones_col, sb_id)

        QT = 4                      # tiles batched through one MLP pass
        for q in range(NT // QT):
            mxq = sm.tile([P, QT * 48], BF16, tag="mxq")
            avq = sm.tile([P, QT * C], BF16, tag="avq")
            for ti in range(QT):
                t = q * QT + ti
                # one DMA: 8 transposed chunks side by side (127, 8*128)
                xgt = xs.tile([GROWS, NG * P], BF16, tag="xgt")
                nc.sync.dma_start(
                    out=xgt[:, :],
                    in_=xg[t * GROWS:(t + 1) * GROWS, :])

                # conv + avg into one PSUM tile: [0:945) conv, [945:990) avg
                tp = ps.tile([P, CL + C], F32, tag="tp")
                for g, (rbase, rdata, cbase, nch) in enumerate(geo):
                    lhs = xgt[0:rdata + 1, g * P:(g + 1) * P]
                    nc.tensor.matmul(
                        tp[:, cbase:cbase + nch * L], lhs,
                        sb_cb[0:rdata + 1, cbase:cbase + nch * L],
                        start=True, stop=True)
                    cav = sum(len(ch) for ch in GROUPS[:g])
                    nc.tensor.matmul(tp[:, CL + cav:CL + cav + nch], lhs,
                                     sb_cb[0:rdata + 1, CL + cav:CL + cav + nch],
                                     start=True, stop=True)

                # avg copy (ACT); max-pool via three PSUM reduces (DVE)
                nc.scalar.activation(out=avq[:, ti * C:(ti + 1) * C],
                                     in_=tp[:, CL:CL + C], func=AF.Copy)
                nc.vector.tensor_reduce(
                    out=mxq[:, ti * 48:ti * 48 + C],
                    in_=_ap(tp[:, 0:CL], [[L, C], [1, L]]),
                    axis=AX.X, op=ALU.max)

            # quad MLP, branches merged as extra columns: one
            # matmul/tanh chain covers 4 tiles x {max, avg} as (.., 1024)
            hTq_ps = psm.tile([C, 2 * QT * P], BF16, tag="hTq")
            for ti in range(QT):
                nc.tensor.transpose(hTq_ps[:, ti * P:(ti + 1) * P],
                                    mxq[:, ti * 48:ti * 48 + C], sb_id)
                nc.tensor.transpose(
                    hTq_ps[:, (QT + ti) * P:(QT + ti + 1) * P],
                    avq[:, ti * C:(ti + 1) * C], sb_id)
            hTq = hw.tile([C, 2 * QT * P], BF16, tag="hTq")
            nc.vector.tensor_copy(out=hTq, in_=hTq_ps)
            pmlp = psm.tile([C, 2 * QT * P], F32, tag="pmlp")
            H = QT * P                      # 512-col halves: one PSUM bank
            nc.tensor.matmul(pmlp[0:25, 0:H], sb_w1a, hTq[:, 0:H],
                             start=True, stop=True)
            nc.tensor.matmul(pmlp[0:25, H:2 * H], sb_w1a, hTq[:, H:2 * H],
                             start=True, stop=True)
            t1Tq = hw.tile([25, 2 * QT * P], BF16, tag="t1Tq")
            nc.scalar.activation(out=t1Tq, in_=pmlp[0:25, :], func=AF.Tanh,
                                 bias=sb_b1c)
            nc.tensor.matmul(pmlp[:, 0:H], sb_w2a, t1Tq[:, 0:H],
                             start=True, stop=True)
            nc.tensor.matmul(pmlp[:, H:2 * H], sb_w2a, t1Tq[:, H:2 * H],
                             start=True, stop=True)
            zTq = hw.tile([C, 2 * QT * P], BF16, tag="zTq")
            nc.scalar.activation(out=zTq, in_=pmlp, func=AF.Tanh, bias=sb_b2c)

            # sum the max/avg halves; transpose back per tile; softmax
            zTsum = hw.tile([C, QT * P], BF16, tag="zTsum")
            nc.vector.tensor_add(out=zTsum, in0=zTq[:, 0:QT * P],
                                 in1=zTq[:, QT * P:2 * QT * P])
            zsq = psm.tile([P, QT * 48], BF16, tag="zsq")
            gtq = sm.tile([P, QT * 48], F16, tag="gtq")
            for ti in range(QT):
                t = q * QT + ti
                nc.tensor.transpose(zsq[:, ti * 48:ti * 48 + C],
                                    zTsum[:, ti * P:(ti + 1) * P],
                                    sb_cb[0:C, CL + 2 * C + 25:CL + 2 * C + 25 + C])
                eg = sm.tile([P, C], F16, tag=f"eg{ti}")
                ssum = sm.tile([P, 1], F32, tag=f"ssum{ti}")
                nc.scalar.activation(out=eg, in_=zsq[:, ti * 48:ti * 48 + C],
                                     func=AF.Exp, accum_out=ssum)
                rs = sm.tile([P, 1], F32, tag=f"rs{ti}")
                nc.vector.reciprocal(out=rs, in_=ssum)
                nc.vector.tensor_scalar_mul(out=gtq[:, ti * 48:ti * 48 + C],
                                            in0=eg, scalar1=rs)
            nc.scalar.dma_start(
                out=_dram_ap(gate_o, q * QT * P * C,
                             [[C, P], [P * C, QT], [1, C]]),
                in_=_ap(gtq[:, 0:], [[48, QT], [1, C]]))
    legalize_sync_waits(nc)
    return nc


def build_attn_program(dj):
    """Rank-1 attention via the polynomial trick.  The host ships, per
    row and in l-major fp16 (element (l,e) at l*22+e): u' = x_sel/2
    (halved so fp16 power sums cannot overflow), kap = a*x_sel + g, and
    the selected gate row.  dj[j] = cheb_j * 2^j are shared step
    immediates.

    den and the gated numerator M = gp*num/2 + gq*den (gp = 4P_e*gate,
    gq = 2Q_e*gate) are evaluated jointly by one Horner pass over a
    duplicated-expert axis eh=44: the step-j coefficient is the
    contiguous 44-wide slice [dj*S'_j | dj*(gp*S'_{j+1}+gq*S'_j)] of a
    prebuilt stack, broadcast over l; then 2*A = M/den and A*x follow.
    Only plain TensorScalarPtr/TensorCopy get DVE 2x/4x modes and walrus
    limits TSP to 2 free dims, so the hot loop is fp16 InstTensorTensor
    (2x_1p) with 3-free-dim access patterns; power sums use a pairwise
    tree over l on an (l, j, e)-interleaved power stack."""
    nc = bass.Bass()
    W3 = 2 * EL + E
    xsg = nc.declare_dram_parameter("xsg", [BC, W3], F16, isOutput=False)
    # packed broadcast constants [djv | p2v | qvv]
    NCC = (J + 1) * 2 * E + 2 * E
    cstc = nc.declare_dram_parameter("cstc", [NCC], F16, isOutput=False)
    o16 = nc.declare_dram_parameter("o16", [BC, 2 * EL], F16, isOutput=True)

    NJ = J + 1        # powers u'^1..u'^{J+1}
    ROW = NJ * E      # one l-row of the interleaved power stack

    with tile.TileContext(nc) as tc, ExitStack() as ctx:
        singles = ctx.enter_context(tc.tile_pool(name="singles", bufs=1))
        pstk = ctx.enter_context(tc.tile_pool(name="pstk", bufs=5))
        trp = ctx.enter_context(tc.tile_pool(name="trp", bufs=5))
        kp = ctx.enter_context(tc.tile_pool(name="kp", bufs=5))
        hp = ctx.enter_context(tc.tile_pool(name="hp", bufs=5))
        op = ctx.enter_context(tc.tile_pool(name="op", bufs=5))

        base = cstc[:]
        cB = singles.tile([P, NCC], F16, name="bc_cstc")
        nc.gpsimd.dma_start(
            out=cB, in_=bass.AP(tensor=base.tensor, offset=base.offset,
                                ap=[[0, P], [1, NCC]]))
        djB = cB[:, 0:(J + 1) * 2 * E]
        p2B = cB[:, NCC - 2 * E:NCC - E]
        qB = cB[:, NCC - E:NCC]

        for t in range(NT):
            ug = kp.tile([P, W3], F16, tag="ug")     # [u' | kap | gate_sel]
            nc.sync.dma_start(out=ug, in_=xsg[t * P:(t + 1) * P, :])
            u0 = ug[:, 0:EL]
            kapv = ug[:, EL:2 * EL]
            gst = ug[:, 2 * EL:W3]

            # interleaved power stack (l, j, e); ACT copies u' to slot 1
            pst = pstk.tile([P, L * ROW], F16, tag="pst")

            def slot(j):          # (l, e) view of power j
                return _ap(pst[:, (j - 1) * E:], [[ROW, L], [1, E]])

            if t == 0:
                nc.vector.tensor_copy(out=slot(1), in_=u0)
            else:
                nc.scalar.activation(out=slot(1), in_=u0, func=AF.Copy)

            # powers u'^2..u'^{J+1}: squares on ACT, odd muls on DVE/Pool
            nc.scalar.activation(out=slot(2), in_=u0, func=AF.Square)
            nc.vector.tensor_mul(out=slot(3), in0=slot(2), in1=slot(1))
            nc.scalar.activation(out=slot(4), in_=slot(2), func=AF.Square)
            if NJ >= 5:
                nc.gpsimd.tensor_mul(out=slot(5), in0=slot(3), in1=slot(2))
            if NJ >= 6:
                nc.scalar.activation(out=slot(6), in_=slot(3), func=AF.Square)
            if NJ >= 7:
                nc.gpsimd.tensor_mul(out=slot(7), in0=slot(6), in1=slot(1))

            # pairwise tree over l; (j,e) stays contiguous throughout, so
            # the final step writes S'_1.. straight into the S-stack
            Sp = trp.tile([P, (NJ + 1) * E], F16, tag="Sp")
            nc.gpsimd.memset(Sp[:, 0:E], float(L))
            t1 = trp.tile([P, 10 * ROW], F16, tag="t1")
            t2 = trp.tile([P, 5 * ROW], F16, tag="t2")
            t3 = trp.tile([P, 2 * ROW], F16, tag="t3")
            t4 = trp.tile([P, ROW], F16, tag="t4")
            t5 = trp.tile([P, ROW], F16, tag="t5")

            pR = lambda l0, n: _ap(pst[:, l0 * ROW:], [[ROW, n], [1, ROW]])
            tR = lambda tl, l0, n: _ap(tl[:, l0 * ROW:], [[ROW, n], [1, ROW]])
            nc.vector.tensor_add(out=tR(t1, 0, 10), in0=pR(0, 10),
                                 in1=pR(10, 10))
            nc.vector.tensor_add(out=tR(t2, 0, 5), in0=tR(t1, 0, 5),
                                 in1=tR(t1, 5, 5))
            nc.vector.tensor_add(out=tR(t3, 0, 2), in0=tR(t2, 0, 2),
                                 in1=tR(t2, 2, 2))
            nc.gpsimd.tensor_add(out=tR(t4, 0, 1), in0=tR(t3, 0, 1),
                                 in1=tR(t3, 1, 1))
            nc.gpsimd.tensor_add(out=tR(t5, 0, 1), in0=tR(t4, 0, 1),
                                 in1=tR(t2, 4, 1))
            nc.gpsimd.tensor_add(out=_ap(Sp[:, E:], [[1, ROW]]),
                                 in0=tR(t5, 0, 1), in1=pR(20, 1))

            # coefficient pairs: [dj*S'_j | dj*(gp*S'_{j+1} + gq*S'_j)]
            gp = kp.tile([P, E], F16, tag="gp")
            nc.gpsimd.tensor_mul(out=gp, in0=gst, in1=p2B)
            gq = kp.tile([P, E], F16, tag="gq")
            nc.gpsimd.tensor_mul(out=gq, in0=gst, in1=qB)
            SS = trp.tile([P, (J + 1) * 2 * E], F16, tag="SS")
            tq = trp.tile([P, (J + 1) * E], F16, tag="tq")
            wJ = lambda tl, off: _ap(tl[:, off:], [[E, J + 1], [1, E]])
            wS = lambda off: _ap(SS[:, off:], [[2 * E, J + 1], [1, E]])
            wD = lambda off: _ap(djB[:, off:], [[2 * E, J + 1], [1, E]])
            gpB = _ap(gp[:, 0:E], [[0, J + 1], [1, E]])
            gqB = _ap(gq[:, 0:E], [[0, J + 1], [1, E]])
            nc.vector.tensor_mul(out=wS(0), in0=wJ(Sp, 0), in1=wD(0))
            nc.vector.tensor_mul(out=wS(E), in0=wJ(Sp, E), in1=gpB)
            nc.vector.tensor_mul(out=wJ(tq, 0), in0=wJ(Sp, 0), in1=gqB)
            nc.vector.tensor_add(out=wS(E), in0=wS(E), in1=wJ(tq, 0))
            nc.vector.tensor_mul(out=wS(E), in0=wS(E), in1=wD(E))

            # joint Horner on [den | M] with 3-free-dim TT ops
            ra = hp.tile([P, 2 * EL], F16, tag="ra")
            rb = hp.tile([P, 2 * EL], F16, tag="rb")
            f44 = lambda tl: _ap(tl[:, 0:], [[EL, 2], [E, L], [1, E]])
            cj = lambda j: _ap(SS[:, j * 2 * E:], [[E, 2], [0, L], [1, E]])
            kB = _ap(ug[:, EL:], [[0, 2], [E, L], [1, E]])
            cur, other = ra, rb
            nc.vector.tensor_mul(out=f44(cur), in0=cj(J), in1=kB)
            for j in range(J - 1, -1, -1):
                nc.vector.tensor_add(out=f44(other), in0=cj(j), in1=f44(cur))
                cur, other = other, cur
                if j > 0:
                    nc.vector.tensor_mul(out=f44(other), in0=f44(cur), in1=kB)
                    cur, other = other, cur

            # 2*A = M/den (host halves A on the way out); A*x = 2A * u'
            rd32 = op.tile([P, EL], F32, tag="rd32")
            nc.vector.reciprocal(out=rd32, in_=cur[:, 0:EL])
            ot = op.tile([P, 2 * EL], F16, tag="ot")
            eng = nc.vector if t == NT - 1 else nc.gpsimd
            eng.tensor_mul(out=ot[:, 0:EL], in0=cur[:, EL:2 * EL], in1=rd32)
            eng.tensor_mul(out=ot[:, EL:2 * EL], in0=ot[:, 0:EL], in1=u0)
            nc.scalar.dma_start(out=o16[t * P:(t + 1) * P, :], in_=ot)
    legalize_sync_waits(nc)
    return nc


def _gate_params(inputs):
    gc_w, gc_b = inputs["gc_w"], inputs["gc_b"]
    wbar = gc_w.mean(0)
    bbar = gc_b.mean()
    NCB = CL + C + 25 + C + P
    cstb = np.zeros((P, NCB), np.float32)
    cb = 0
    for g, chans in enumerate(GROUPS):
        nch = len(chans)
        for k, c in enumerate(chans):
            cstb[k * L:(k + 1) * L, cb + k * L:cb + (k + 1) * L] = gc_w.T
            cstb[k * L:(k + 1) * L, CL + c] = wbar
            cstb[nch * L, CL + c] = bbar
        cstb[nch * L, cb:cb + nch * L] = np.tile(gc_b, nch)
        cb += nch * L
    cstb[0:C, CL + C:CL + C + 25] = inputs["w1"].T
    cstb[0:25, CL + C + 25:CL + 2 * C + 25] = inputs["w2"].T
    cstb[:, CL + 2 * C + 25:NCB] = np.eye(P)
    cstf = np.zeros((C, 2), np.float32)
    cstf[0:25, 0] = inputs["b1"]
    cstf[0:C, 1] = inputs["b2"]
    return cstb.astype(ml_dtypes.bfloat16), cstf


_CACHE = {}


def kernel(**inputs):
    inputs = {k: np.ascontiguousarray(np.asarray(v)) for k, v in inputs.items()}
    x = inputs["x"].astype(np.float32)              # (B, C, L)
    bf = ml_dtypes.bfloat16
    cores = list(range(NCORES))

    # ---- launch 1: gate -------------------------------------------------
    cstb, cstf = _gate_params(inputs)
    # host-marshaled transposed x: 8 chunks of (6ch x 21 + ones row) x B
    xt = x.reshape(B, CL).T                          # (945, B)
    xgc = np.zeros((NG * GROWS, B), np.float32)
    cb = 0
    for g, chans in enumerate(GROUPS):
        nch = len(chans)
        xgc[g * GROWS:g * GROWS + nch * L] = xt[cb:cb + nch * L]
        xgc[g * GROWS + nch * L] = 1.0
        cb += nch * L
    # tile-major repack: per 128-row batch tile, the 8 chunks side by side
    # so each tile load is one contiguous (127, 1024) DMA
    xg = np.ascontiguousarray(
        xgc.reshape(NG, GROWS, NCORES, NT, P)       # (g, r, core, t, p)
        .transpose(2, 3, 1, 0, 4)                    # (core, t, r, g, p)
        .reshape(NCORES, NT * GROWS, NG * P)).astype(bf)

    if "gate" not in _CACHE:
        _CACHE["gate"] = build_gate_program()
    nc1 = _CACHE["gate"]
    maps1 = [{"xg": xg[i], "cstb": cstb, "cstf": cstf} for i in cores]
    r1 = run_bass_kernel_spmd(nc1, maps1, cores).results
    gate16 = np.concatenate([np.asarray(r["gate"]) for r in r1], 0)  # (B,45)

    # ---- routing (host-mediated all-reduce) -----------------------------
    mean_gate = gate16.astype(np.float64).mean(0)
    sel = np.sort(np.argsort(-mean_gate, kind="stable")[:E])

    # ---- launch 2: attention -------------------------------------------
    wq, bq = inputs["wq"], inputs["bq"]
    wk, bk = inputs["wk"], inputs["bk"]
    wv, bv = inputs["wv"], inputs["bv"]
    wo, bo = inputs["wo"], inputs["bo"]
    alpha = (wq * wk).sum(1).astype(np.float64)
    gamma = (bq * wk).sum(1).astype(np.float64)
    pv = (wo * wv).sum(1).astype(np.float64)
    qv = ((wo * bv).sum(1) + bo).astype(np.float64)

    xsel = x[:, sel, :]                              # (B, E, L)
    umax = float(np.abs(xsel).max())
    zm = (np.abs(alpha).max() * umax + np.abs(gamma).max()) * umax
    cheb = np.polynomial.chebyshev.Chebyshev.interpolate(
        np.exp, J, domain=[-zm, zm])
    dc = cheb.convert(kind=np.polynomial.Polynomial).coef
    dj = [float(dc[j] * (2.0 ** j)) for j in range(J + 1)]

    key = tuple(np.round(dj, 12))
    if _CACHE.get("attn_key") != key:
        _CACHE["attn"] = build_attn_program(dj)
        _CACHE["attn_key"] = key
    nc2 = _CACHE["attn"]

    xsg = np.empty((B, 2 * EL + E), np.float16)  # [u' | kap | gate] l-major
    xlm = np.ascontiguousarray(xsel.transpose(0, 2, 1).astype(np.float32))
    xsg[:, :EL] = (xlm * np.float32(0.5)).reshape(B, EL)
    xsg[:, EL:2 * EL] = (xlm * alpha.astype(np.float32)[None, None, :]
                         + gamma.astype(np.float32)[None, None, :]
                         ).reshape(B, EL)
    xsg[:, 2 * EL:] = gate16[:, sel]
    cstc = np.concatenate([
        np.repeat(np.asarray(dj), 2 * E).astype(np.float16),
        (4 * pv).astype(np.float16),
        (2 * qv).astype(np.float16)]).astype(np.float16)
    maps2 = [{"xsg": xsg[i * BC:(i + 1) * BC], "cstc": cstc}
             for i in cores]
    r2 = run_bass_kernel_spmd(nc2, maps2, cores).results
    o16 = np.concatenate([np.asarray(r["o16"]) for r in r2], 0)  # (B, 924)

    # ---- host unshard / scatter (device emits 2*A and A*x) -------------
    at = (o16[:, :EL].astype(np.float32) * 0.5).reshape(
        B, L, E).transpose(0, 2, 1)
    gt = o16[:, EL:].astype(np.float32).reshape(B, L, E).transpose(0, 2, 1)
    cols = (np.repeat(sel * L, L) + np.tile(np.arange(L), E))
    A_full = np.zeros((B, CL), np.float32)
    G_full = np.zeros((B, CL), np.float32)
    A_full[:, cols] = at.reshape(B, EL)
    G_full[:, cols] = gt.reshape(B, EL)
    return G_full, A_full
